# revision 7
# baseline (speedup 1.0000x reference)
"""PointNet feature extractor on 8 Trainium2 NeuronCores (Bass/Tile).

Problem: x (8, 16384, 3) -> 3x [conv1d(k=1) + sync-BN (+ReLU)] ->
global max-pool -> out (8, 1088, 16384) where rows 0:1024 are the
broadcast global feature and rows 1024:1088 are the (transposed) local
(layer-0) features.

Sharding: data-parallel over batch, 1 batch per core.  BN statistics
(per-channel sum / sum-of-squares) are AllReduced across the 8 cores.

Key algebraic facts used:
  * The conv biases b0/b1/b2 cancel exactly inside training-mode BN
    (mean subtraction), so they are never loaded.
  * BN is a per-channel affine y = scale*u + shift with
    scale = gamma * rsqrt(var+eps) > 0 (gamma = 1 in this problem), so
    max_n BN(u) = BN(max_n u): we never materialize the normalized
    layer-2 activations, just per-channel max of the pre-BN values.
  * mean of h2 = W2 @ (AllReduce sum of h1) / N_total, so only
    sum-of-squares of h2 needs a per-channel-chunk reduction pass.

Performance structure (measured via REPS-slope on hardware):
  * All output DMA (71 MB/core) is split column-wise across the three
    DMA-capable queues (SP HWDGE / ACT HWDGE / Pool SWDGE); a single
    queue sustains only ~29 GB/s and was 2.45 ms of a 2.77 ms body.
  * AllReduces cost ~29 us each and paced the layer-2 pipeline, so the
    8 per-chunk stat reductions are batched in pairs and the h1 row-sum
    rides along with the first pair (6 collectives total instead of 11).
  * The layer-2 matmuls (85% of FLOPs) run in bf16 (tolerance is 2e-2;
    bf16 contributes ~2e-4 absmax-relative error).
"""

import functools
import numpy as np

B = 8
N = 16384          # points per batch == points per core (1 batch / core)
NTOT = B * N       # BN statistics population size
EPS = 1e-5
NCORES = 8
NCHUNK = 2048      # PSUM evacuation chunk (4 banks)
T = N // NCHUNK    # 8 chunks
BATCHES = [(0, 2), (2, 4), (4, 7), (7, 8)]  # layer-2 chunk batches per AllReduce


def _body(nc, tc, io):
    from concourse import mybir

    f32 = mybir.dt.float32
    bf = mybir.dt.bfloat16
    AF = mybir.ActivationFunctionType
    OP = mybir.AluOpType
    AX = mybir.AxisListType
    RG = [list(range(NCORES))]

    out = io["out"]

    with (
        tc.tile_pool(name="singles", bufs=1) as singles,
        tc.tile_pool(name="big", bufs=1) as big,
        tc.tile_pool(name="scr", bufs=2) as scr,
        tc.tile_pool(name="stat", bufs=1) as stat,
        tc.tile_pool(name="statl", bufs=2) as statl,
        tc.tile_pool(name="bcp", bufs=2) as bcp,
        tc.tile_pool(name="psum", bufs=2, space="PSUM") as psum,
        tc.tile_pool(name="dram", bufs=1, space="DRAM") as dram,
    ):
        # ---------------- load inputs ----------------
        # xT shares its SBUF slot with u1 (phase B) via the pool tag: xT is
        # dead once the layer-0 matmuls are done.
        sb_xT = big.tile([3, N], f32, tag="u1x")
        nc.gpsimd.dma_start(sb_xT[:], io["xT"])
        sb_w0t = singles.tile([3, 64], f32)
        nc.gpsimd.dma_start(sb_w0t[:], io["w0t"])
        sb_w1t = singles.tile([64, 128], f32)
        nc.gpsimd.dma_start(sb_w1t[:], io["w1t"])
        sb_w2t = singles.tile([128, 1024], f32)
        nc.gpsimd.dma_start(sb_w2t[:], io["w2t"])
        sb_g0 = singles.tile([64, 2], f32)
        nc.gpsimd.dma_start(sb_g0[:], io["g0be0"])
        sb_g1 = singles.tile([128, 2], f32)
        nc.gpsimd.dma_start(sb_g1[:], io["g1be1"])
        sb_g2 = singles.tile([128, 8], f32)
        nc.gpsimd.dma_start(sb_g2[:], io["g2p"])
        sb_be2 = singles.tile([128, 8], f32)
        nc.gpsimd.dma_start(sb_be2[:], io["be2p"])

        sb_eps = singles.tile([128, 1], f32)
        nc.vector.memset(sb_eps[:], EPS)

        # bf16 copy of W2 for the layer-2 matmuls
        sb_w2b = singles.tile([128, 1024], bf)
        nc.scalar.activation(sb_w2b[:], sb_w2t[:], AF.Copy)

        def allreduce(src_ap, p, f, tag):
            """AllReduce add of an SBUF [p, f] region; returns SBUF tile.

            The collective output must live in the Shared DRAM scratchpad —
            with a Local (or pool) output the exec unit crashes under this
            runtime (NRT_EXEC_UNIT_UNRECOVERABLE).
            """
            d_in = dram.tile([p, f], f32, tag=f"ar_in_{tag}")
            d_out = nc.dram_tensor(
                f"cc_out_{tag}", [p, f], f32, kind="Internal", addr_space="Shared"
            )
            nc.gpsimd.dma_start(d_in[:], src_ap)
            nc.gpsimd.collective_compute(
                "AllReduce",
                OP.add,
                replica_groups=RG,
                ins=[d_in[:].opt()],
                outs=[d_out.ap().opt()],
            )
            red = stat.tile([p, f], f32, tag=f"ar_red_{tag}")
            nc.gpsimd.dma_start(red[:], d_out.ap())
            return red

        def bn_affine(red, gbe, p, tag):
            """From AllReduced [p,2] (sum, sumsq) + gamma/beta [p,2] make
            (scale, shift) [p,1] tiles: y = scale*u + shift."""
            w = stat.tile([p, 6], f32, tag=f"bnw_{tag}")
            # w0 = mean, w1 = E[u^2]
            nc.scalar.mul(w[:, 0:2], red[:, 0:2], 1.0 / NTOT)
            # w2 = mean^2 ; w3 = var = E[u^2] - mean^2
            nc.vector.tensor_mul(w[:, 2:3], w[:, 0:1], w[:, 0:1])
            nc.vector.tensor_sub(w[:, 3:4], w[:, 1:2], w[:, 2:3])
            # w4 = sqrt(var + eps)
            nc.scalar.activation(w[:, 4:5], w[:, 3:4], AF.Sqrt, bias=sb_eps[:p, 0:1])
            # w5 = rstd
            nc.vector.reciprocal(w[:, 5:6], w[:, 4:5])
            sc = stat.tile([p, 2], f32, tag=f"bnsc_{tag}")
            # scale = gamma * rstd
            nc.vector.tensor_mul(sc[:, 0:1], gbe[:, 0:1], w[:, 5:6])
            # shift = beta - mean * scale
            nc.vector.tensor_mul(sc[:, 1:2], w[:, 0:1], sc[:, 0:1])
            nc.vector.tensor_sub(sc[:, 1:2], gbe[:, 1:2], sc[:, 1:2])
            return sc

        # ---------------- layer 0 ----------------
        u0 = big.tile([64, N], f32)
        s0p = stat.tile([64, T], f32)
        q0p = stat.tile([64, T], f32)
        for t in range(T):
            pa = psum.tile([128, NCHUNK], f32, tag="mm")
            for k in range(4):
                nc.tensor.matmul(
                    pa[0:64, k * 512:(k + 1) * 512],
                    sb_w0t[:],
                    sb_xT[:, t * NCHUNK + k * 512: t * NCHUNK + (k + 1) * 512],
                )
            # evacuate + per-chunk sum (ACT), per-chunk sumsq (ACT Square)
            nc.scalar.activation(
                u0[:, t * NCHUNK:(t + 1) * NCHUNK], pa[0:64, :], AF.Copy,
                accum_out=s0p[:, t:t + 1],
            )
            # sumsq on DVE (idle during layers 0/1) so ACT only evacuates
            sq = scr.tile([128, NCHUNK], f32, tag="sq")
            nc.vector.tensor_mul(
                sq[0:64, :], u0[:, t * NCHUNK:(t + 1) * NCHUNK],
                u0[:, t * NCHUNK:(t + 1) * NCHUNK],
            )
            nc.vector.reduce_sum(q0p[:, t:t + 1], sq[0:64, :], axis=AX.X)
        ar0 = stat.tile([64, 2], f32)
        nc.vector.reduce_sum(ar0[:, 0:1], s0p[:], axis=AX.X)
        nc.vector.reduce_sum(ar0[:, 1:2], q0p[:], axis=AX.X)
        red0 = allreduce(ar0[:], 64, 2, "bn0")
        sc0 = bn_affine(red0, sb_g0, 64, "bn0")

        # normalize + relu in place: local = relu(scale*u0 + shift)
        for c in range(2):
            h = N // 2
            nc.scalar.activation(
                u0[:, c * h:(c + 1) * h], u0[:, c * h:(c + 1) * h],
                AF.Relu, bias=sc0[:, 1:2], scale=sc0[:, 0:1],
            )
        local = u0
        # local features -> output rows 1024:1088; issued early so the DMA
        # queues have work while layer 1/2 compute runs
        nc.sync.dma_start(out[1024:1088, 0:8192], local[:, 0:8192])
        nc.scalar.dma_start(out[1024:1088, 8192:16384], local[:, 8192:16384])

        # ---------------- layer 1 ----------------
        u1 = big.tile([128, N], f32, tag="u1x")
        s1p = stat.tile([128, T], f32)
        q1p = stat.tile([128, T], f32)
        for t in range(T):
            pb = psum.tile([128, NCHUNK], f32, tag="mm")
            for k in range(4):
                nc.tensor.matmul(
                    pb[:, k * 512:(k + 1) * 512],
                    sb_w1t[:],
                    local[:, t * NCHUNK + k * 512: t * NCHUNK + (k + 1) * 512],
                )
            nc.scalar.activation(
                u1[:, t * NCHUNK:(t + 1) * NCHUNK], pb[:], AF.Copy,
                accum_out=s1p[:, t:t + 1],
            )
            sq = scr.tile([128, NCHUNK], f32, tag="sq")
            nc.vector.tensor_mul(
                sq[:], u1[:, t * NCHUNK:(t + 1) * NCHUNK],
                u1[:, t * NCHUNK:(t + 1) * NCHUNK],
            )
            nc.vector.reduce_sum(q1p[:, t:t + 1], sq[:], axis=AX.X)
        ar1 = stat.tile([128, 2], f32)
        nc.vector.reduce_sum(ar1[:, 0:1], s1p[:], axis=AX.X)
        nc.vector.reduce_sum(ar1[:, 1:2], q1p[:], axis=AX.X)
        red1 = allreduce(ar1[:], 128, 2, "bn1")
        sc1 = bn_affine(red1, sb_g1, 128, "bn1")

        # normalize + relu, casting to bf16 for the layer-2 matmuls
        h1 = big.tile([128, N], bf, tag="h1b")
        for c in range(2):
            h = N // 2
            nc.scalar.activation(
                h1[:, c * h:(c + 1) * h], u1[:, c * h:(c + 1) * h],
                AF.Relu, bias=sc1[:, 1:2], scale=sc1[:, 0:1],
            )

        # sum of h1 over points (for layer-2 BN mean via W2 @ sum); rides
        # along with the first layer-2 stat AllReduce
        sh1 = stat.tile([128, 1], f32)
        nc.vector.reduce_sum(sh1[:], h1[:], axis=AX.X)

        # ---------------- layer 2 + max pool + output ----------------
        # Chunk batches per AllReduce: [2,2,3,1] — early batches small enough
        # to start the 8 MB/chunk writes promptly, last batch minimal so the
        # final collective only gates an 8 MB write tail.
        m2 = stat.tile([128, 8], f32)
        for p_, (j0_, j1_) in enumerate(BATCHES):
            js = list(range(j0_, j1_))
            mxp = statl.tile([128, 3 * T], f32, tag="mxp")
            q2p = statl.tile([128, 3 * T], f32, tag="q2p")
            for ji, j in enumerate(js):
                for t in range(T):
                    pc = psum.tile([128, NCHUNK], f32, tag="mm")
                    for k in range(4):
                        nc.tensor.matmul(
                            pc[:, k * 512:(k + 1) * 512],
                            sb_w2b[:, j * 128:(j + 1) * 128],
                            h1[:, t * NCHUNK + k * 512: t * NCHUNK + (k + 1) * 512],
                        )
                    sq = scr.tile([128, NCHUNK], f32, tag="sq")
                    nc.scalar.activation(
                        sq[:], pc[:], AF.Square,
                        accum_out=q2p[:, ji * T + t:ji * T + t + 1],
                    )
                    nc.vector.reduce_max(
                        mxp[:, ji * T + t:ji * T + t + 1], pc[:], axis=AX.X)
            # batched stats: [sh1 (first batch only) | q2 sums for the batch]
            extra = 1 if p_ == 0 else 0
            nb = extra + len(js)
            stj = statl.tile([128, 4], f32, tag="stj")
            if extra:
                nc.vector.tensor_copy(stj[:, 0:1], sh1[:])
            for ji in range(len(js)):
                nc.vector.reduce_sum(
                    stj[:, extra + ji:extra + ji + 1],
                    q2p[:, ji * T:(ji + 1) * T], axis=AX.X)
            red = allreduce(stj[:, 0:nb], 128, nb, f"q2_{p_}")
            if extra:
                # mean2[:, j] = (W2 @ sum_h1)_chunk_j / NTOT
                sh1hat = stat.tile([128, 1], f32)
                nc.vector.tensor_copy(sh1hat[:], red[:, 0:1])
                for j in range(8):
                    pm = psum.tile([128, 1], f32, tag="mm")
                    nc.tensor.matmul(
                        pm[:], sb_w2t[:, j * 128:(j + 1) * 128], sh1hat[:])
                    nc.scalar.mul(m2[:, j:j + 1], pm[:], 1.0 / NTOT)
            # affine-transform the channel maxes + write 8 MB per chunk
            for ji, j in enumerate(js):
                mx = statl.tile([128, 1], f32, tag="mx")
                nc.vector.reduce_max(mx[:], mxp[:, ji * T:(ji + 1) * T], axis=AX.X)
                wv = statl.tile([128, 4], f32, tag="wv")
                # var = E[h2^2] - mean^2 ; rstd
                nc.scalar.mul(wv[:, 0:1], red[:, extra + ji:extra + ji + 1], 1.0 / NTOT)
                nc.vector.tensor_mul(wv[:, 1:2], m2[:, j:j + 1], m2[:, j:j + 1])
                nc.vector.tensor_sub(wv[:, 1:2], wv[:, 0:1], wv[:, 1:2])
                nc.scalar.activation(wv[:, 2:3], wv[:, 1:2], AF.Sqrt, bias=sb_eps[:, 0:1])
                nc.vector.reciprocal(wv[:, 3:4], wv[:, 2:3])
                # scale2 = g2*rstd ; shift2 = be2 - mean*scale2 ;
                # gfeat = scale2*max + shift2
                sc2 = statl.tile([128, 3], f32, tag="sc2")
                nc.vector.tensor_mul(sc2[:, 0:1], sb_g2[:, j:j + 1], wv[:, 3:4])
                nc.vector.tensor_mul(sc2[:, 1:2], m2[:, j:j + 1], sc2[:, 0:1])
                nc.vector.tensor_sub(sc2[:, 1:2], sb_be2[:, j:j + 1], sc2[:, 1:2])
                gf = statl.tile([128, 1], f32, tag="gf")
                nc.vector.tensor_scalar(
                    out=gf[:], in0=mx[:], scalar1=sc2[:, 0:1],
                    scalar2=sc2[:, 1:2], op0=OP.mult, op1=OP.add,
                )
                # broadcast gfeat along the free dim; the 8 MB write is split
                # column-wise over the three DMA queues (rotating per j)
                bc = bcp.tile([128, NCHUNK], f32, tag="bc")
                nc.vector.tensor_copy(bc[:], gf[:, 0:1].to_broadcast([128, NCHUNK]))
                engs = [nc.sync, nc.scalar, nc.gpsimd]
                splits = [(0, 3), (3, 6), (6, 8)]
                for qi, (a, b_) in enumerate(splits):
                    eng = engs[(qi + j) % 3]
                    srcq = bc[:].unsqueeze(1).broadcast_to([128, b_ - a, NCHUNK])
                    eng.dma_start(
                        out[j * 128:(j + 1) * 128, a * NCHUNK:b_ * NCHUNK], srcq)


@functools.lru_cache(maxsize=1)
def build_program():
    import concourse.bacc as bacc
    import concourse.tile as tile
    from concourse import mybir

    f32 = mybir.dt.float32
    nc = bacc.Bacc(
        "TRN2", target_bir_lowering=False, debug=False, num_devices=NCORES
    )
    io = {
        "xT": nc.dram_tensor("xT", [3, N], f32, kind="ExternalInput").ap(),
        "w0t": nc.dram_tensor("w0t", [3, 64], f32, kind="ExternalInput").ap(),
        "w1t": nc.dram_tensor("w1t", [64, 128], f32, kind="ExternalInput").ap(),
        "w2t": nc.dram_tensor("w2t", [128, 1024], f32, kind="ExternalInput").ap(),
        "g0be0": nc.dram_tensor("g0be0", [64, 2], f32, kind="ExternalInput").ap(),
        "g1be1": nc.dram_tensor("g1be1", [128, 2], f32, kind="ExternalInput").ap(),
        "g2p": nc.dram_tensor("g2p", [128, 8], f32, kind="ExternalInput").ap(),
        "be2p": nc.dram_tensor("be2p", [128, 8], f32, kind="ExternalInput").ap(),
        "out": nc.dram_tensor("out", [1088, N], f32, kind="ExternalOutput").ap(),
    }
    with tile.TileContext(nc) as tc:
        _body(nc, tc, io)
    nc.compile()
    return nc


def make_in_maps(x, W0, W1, W2, g0, be0, g1, be1, g2, be2):
    x = np.asarray(x, np.float32)
    shared = {
        "w0t": np.ascontiguousarray(np.asarray(W0, np.float32).T),
        "w1t": np.ascontiguousarray(np.asarray(W1, np.float32).T),
        "w2t": np.ascontiguousarray(np.asarray(W2, np.float32).T),
        "g0be0": np.ascontiguousarray(
            np.stack([np.asarray(g0, np.float32), np.asarray(be0, np.float32)], 1)),
        "g1be1": np.ascontiguousarray(
            np.stack([np.asarray(g1, np.float32), np.asarray(be1, np.float32)], 1)),
        "g2p": np.ascontiguousarray(np.asarray(g2, np.float32).reshape(8, 128).T),
        "be2p": np.ascontiguousarray(np.asarray(be2, np.float32).reshape(8, 128).T),
    }
    return [
        {"xT": np.ascontiguousarray(x[i].T), **shared} for i in range(NCORES)
    ]


def kernel(x, W0, b0, g0, be0, W1, b1, g1, be1, W2, b2, g2, be2):
    """Full inputs in, full output out.  b0/b1/b2 cancel inside BN."""
    from concourse.bass_utils import run_bass_kernel_spmd

    nc = build_program()
    in_maps = make_in_maps(x, W0, W1, W2, g0, be0, g1, be1, g2, be2)
    res = run_bass_kernel_spmd(nc, in_maps, core_ids=list(range(NCORES)))
    return np.stack([res.results[i]["out"] for i in range(NCORES)], axis=0)


# revision 9
# speedup vs baseline: 1.6144x; 1.6144x over previous
"""PointNet feature extractor on 8 Trainium2 NeuronCores (Bass/Tile).

Problem: x (8, 16384, 3) -> 3x [conv1d(k=1) + sync-BN (+ReLU)] ->
global max-pool -> out (8, 1088, 16384) where rows 0:1024 are the
broadcast global feature and rows 1024:1088 are the (transposed) local
(layer-0) features.

Sharding: data-parallel over batch, 1 batch per core.  BN statistics
(per-channel sum / sum-of-squares) are AllReduced across the 8 cores.

Key algebraic facts used:
  * The conv biases b0/b1/b2 cancel exactly inside training-mode BN
    (mean subtraction), so they are never loaded.
  * BN is a per-channel affine y = scale*u + shift with
    scale = gamma * rsqrt(var+eps) > 0 (gamma = 1 in this problem), so
    max_n BN(u) = BN(max_n u): we never materialize the normalized
    layer-2 activations, just per-channel max of the pre-BN values.
  * mean of h2 = W2 @ (AllReduce sum of h1) / N_total, so only
    sum-of-squares of h2 needs a per-channel-chunk reduction pass.

Performance structure (measured via REPS-slope on hardware):
  * All output DMA (71 MB/core) is split column-wise across the three
    DMA-capable queues (SP HWDGE / ACT HWDGE / Pool SWDGE); a single
    queue sustains only ~29 GB/s and was 2.45 ms of a 2.77 ms body.
  * AllReduces cost ~29 us each and paced the layer-2 pipeline, so the
    8 per-chunk stat reductions are batched in pairs and the h1 row-sum
    rides along with the first pair (6 collectives total instead of 11).
  * The layer-2 matmuls (85% of FLOPs) run in bf16 (tolerance is 2e-2;
    bf16 contributes ~2e-4 absmax-relative error).
"""

import functools
import numpy as np

B = 8
N = 16384          # points per batch == points per core (1 batch / core)
NTOT = B * N       # BN statistics population size
EPS = 1e-5
NCORES = 8
NCHUNK = 2048      # PSUM evacuation chunk (4 banks)
T = N // NCHUNK    # 8 chunks
BATCHES = [(0, 2), (2, 4), (4, 7), (7, 8)]  # layer-2 chunk batches per AllReduce


def _body(nc, tc, io):
    from concourse import mybir

    f32 = mybir.dt.float32
    bf = mybir.dt.bfloat16
    AF = mybir.ActivationFunctionType
    OP = mybir.AluOpType
    AX = mybir.AxisListType
    RG = [list(range(NCORES))]

    out = io["out"]

    with (
        tc.tile_pool(name="singles", bufs=1) as singles,
        tc.tile_pool(name="big", bufs=1) as big,
        tc.tile_pool(name="scr", bufs=2) as scr,
        tc.tile_pool(name="stat", bufs=1) as stat,
        tc.tile_pool(name="statl", bufs=2) as statl,
        tc.tile_pool(name="bcp", bufs=2) as bcp,
        tc.tile_pool(name="psum", bufs=2, space="PSUM") as psum,
        tc.tile_pool(name="dram", bufs=1, space="DRAM") as dram,
    ):
        # ---------------- load inputs ----------------
        # xT shares its SBUF slot with u1 (phase B) via the pool tag: xT is
        # dead once the layer-0 matmuls are done.
        sb_xT = big.tile([3, N], f32, tag="u1x")
        nc.gpsimd.dma_start(sb_xT[:], io["xT"])
        sb_w0t = singles.tile([3, 64], f32)
        nc.gpsimd.dma_start(sb_w0t[:], io["w0t"])
        sb_w1t = singles.tile([64, 128], f32)
        nc.gpsimd.dma_start(sb_w1t[:], io["w1t"])
        sb_w2t = singles.tile([128, 1024], f32)
        nc.gpsimd.dma_start(sb_w2t[:], io["w2t"])
        sb_g0 = singles.tile([64, 2], f32)
        nc.gpsimd.dma_start(sb_g0[:], io["g0be0"])
        sb_g1 = singles.tile([128, 2], f32)
        nc.gpsimd.dma_start(sb_g1[:], io["g1be1"])
        sb_g2 = singles.tile([128, 8], f32)
        nc.gpsimd.dma_start(sb_g2[:], io["g2p"])
        sb_be2 = singles.tile([128, 8], f32)
        nc.gpsimd.dma_start(sb_be2[:], io["be2p"])

        sb_eps = singles.tile([128, 1], f32)
        nc.vector.memset(sb_eps[:], EPS)

        # bf16 copy of W2 for the layer-2 matmuls
        sb_w2b = singles.tile([128, 1024], bf)
        nc.scalar.activation(sb_w2b[:], sb_w2t[:], AF.Copy)

        def allreduce(src_ap, p, f, tag):
            """AllReduce add of an SBUF [p, f] region; returns SBUF tile.

            The collective output must live in the Shared DRAM scratchpad —
            with a Local (or pool) output the exec unit crashes under this
            runtime (NRT_EXEC_UNIT_UNRECOVERABLE).
            """
            d_in = dram.tile([p, f], f32, tag=f"ar_in_{tag}")
            d_out = nc.dram_tensor(
                f"cc_out_{tag}", [p, f], f32, kind="Internal", addr_space="Shared"
            )
            nc.gpsimd.dma_start(d_in[:], src_ap)
            nc.gpsimd.collective_compute(
                "AllReduce",
                OP.add,
                replica_groups=RG,
                ins=[d_in[:].opt()],
                outs=[d_out.ap().opt()],
            )
            red = stat.tile([p, f], f32, tag=f"ar_red_{tag}")
            nc.gpsimd.dma_start(red[:], d_out.ap())
            return red

        def bn_affine(red, gbe, p, tag):
            """From AllReduced [p,2] (sum, sumsq) + gamma/beta [p,2] make
            (scale, shift) [p,1] tiles: y = scale*u + shift."""
            w = stat.tile([p, 6], f32, tag=f"bnw_{tag}")
            # w0 = mean, w1 = E[u^2]
            nc.scalar.mul(w[:, 0:2], red[:, 0:2], 1.0 / NTOT)
            # w2 = mean^2 ; w3 = var = E[u^2] - mean^2
            nc.vector.tensor_mul(w[:, 2:3], w[:, 0:1], w[:, 0:1])
            nc.vector.tensor_sub(w[:, 3:4], w[:, 1:2], w[:, 2:3])
            # w4 = sqrt(var + eps)
            nc.scalar.activation(w[:, 4:5], w[:, 3:4], AF.Sqrt, bias=sb_eps[:p, 0:1])
            # w5 = rstd
            nc.vector.reciprocal(w[:, 5:6], w[:, 4:5])
            sc = stat.tile([p, 2], f32, tag=f"bnsc_{tag}")
            # scale = gamma * rstd
            nc.vector.tensor_mul(sc[:, 0:1], gbe[:, 0:1], w[:, 5:6])
            # shift = beta - mean * scale
            nc.vector.tensor_mul(sc[:, 1:2], w[:, 0:1], sc[:, 0:1])
            nc.vector.tensor_sub(sc[:, 1:2], gbe[:, 1:2], sc[:, 1:2])
            return sc

        # ---------------- layer 0 ----------------
        u0 = big.tile([64, N], f32)
        s0p = stat.tile([64, T], f32)
        q0p = stat.tile([64, T], f32)
        for t in range(T):
            pa = psum.tile([128, NCHUNK], f32, tag="mm")
            for k in range(4):
                nc.tensor.matmul(
                    pa[0:64, k * 512:(k + 1) * 512],
                    sb_w0t[:],
                    sb_xT[:, t * NCHUNK + k * 512: t * NCHUNK + (k + 1) * 512],
                )
            # evacuate + per-chunk sum (ACT), per-chunk sumsq (ACT Square)
            nc.scalar.activation(
                u0[:, t * NCHUNK:(t + 1) * NCHUNK], pa[0:64, :], AF.Copy,
                accum_out=s0p[:, t:t + 1],
            )
            # sumsq on DVE (idle during layers 0/1) so ACT only evacuates
            sq = scr.tile([128, NCHUNK], f32, tag="sq")
            nc.vector.tensor_mul(
                sq[0:64, :], u0[:, t * NCHUNK:(t + 1) * NCHUNK],
                u0[:, t * NCHUNK:(t + 1) * NCHUNK],
            )
            nc.vector.reduce_sum(q0p[:, t:t + 1], sq[0:64, :], axis=AX.X)
        ar0 = stat.tile([64, 2], f32)
        nc.vector.reduce_sum(ar0[:, 0:1], s0p[:], axis=AX.X)
        nc.vector.reduce_sum(ar0[:, 1:2], q0p[:], axis=AX.X)
        red0 = allreduce(ar0[:], 64, 2, "bn0")
        sc0 = bn_affine(red0, sb_g0, 64, "bn0")

        # normalize + relu in place: local = relu(scale*u0 + shift).
        # 4 chunks so layer-1 matmuls (and the local write) start after a
        # quarter of the normalization instead of half.
        for c in range(4):
            h = N // 4
            nc.scalar.activation(
                u0[:, c * h:(c + 1) * h], u0[:, c * h:(c + 1) * h],
                AF.Relu, bias=sc0[:, 1:2], scale=sc0[:, 0:1],
            )
        local = u0
        # local features -> output rows 1024:1088; issued early (split over
        # all three DMA queues for load balance) so the queues have work
        # while layer 1/2 compute runs
        nc.sync.dma_start(out[1024:1088, 0:6144], local[:, 0:6144])
        nc.scalar.dma_start(out[1024:1088, 6144:12288], local[:, 6144:12288])
        nc.gpsimd.dma_start(out[1024:1088, 12288:16384], local[:, 12288:16384])

        # ---------------- layer 1 ----------------
        u1 = big.tile([128, N], f32, tag="u1x")
        s1p = stat.tile([128, T], f32)
        q1p = stat.tile([128, T], f32)
        for t in range(T):
            pb = psum.tile([128, NCHUNK], f32, tag="mm")
            for k in range(4):
                nc.tensor.matmul(
                    pb[:, k * 512:(k + 1) * 512],
                    sb_w1t[:],
                    local[:, t * NCHUNK + k * 512: t * NCHUNK + (k + 1) * 512],
                )
            nc.scalar.activation(
                u1[:, t * NCHUNK:(t + 1) * NCHUNK], pb[:], AF.Copy,
                accum_out=s1p[:, t:t + 1],
            )
            sq = scr.tile([128, NCHUNK], f32, tag="sq")
            nc.vector.tensor_mul(
                sq[:], u1[:, t * NCHUNK:(t + 1) * NCHUNK],
                u1[:, t * NCHUNK:(t + 1) * NCHUNK],
            )
            nc.vector.reduce_sum(q1p[:, t:t + 1], sq[:], axis=AX.X)
        ar1 = stat.tile([128, 2], f32)
        nc.vector.reduce_sum(ar1[:, 0:1], s1p[:], axis=AX.X)
        nc.vector.reduce_sum(ar1[:, 1:2], q1p[:], axis=AX.X)
        red1 = allreduce(ar1[:], 128, 2, "bn1")
        sc1 = bn_affine(red1, sb_g1, 128, "bn1")

        # normalize + relu, casting to bf16 for the layer-2 matmuls
        h1 = big.tile([128, N], bf, tag="h1b")
        for c in range(4):
            h = N // 4
            nc.scalar.activation(
                h1[:, c * h:(c + 1) * h], u1[:, c * h:(c + 1) * h],
                AF.Relu, bias=sc1[:, 1:2], scale=sc1[:, 0:1],
            )

        # sum of h1 over points (for layer-2 BN mean via W2 @ sum); rides
        # along with the first layer-2 stat AllReduce
        sh1 = stat.tile([128, 1], f32)
        nc.vector.reduce_sum(sh1[:], h1[:], axis=AX.X)

        # ---------------- layer 2 + max pool + output ----------------
        # Chunk batches per AllReduce: [2,2,3,1] — early batches small enough
        # to start the 8 MB/chunk writes promptly, last batch minimal so the
        # final collective only gates an 8 MB write tail.
        m2 = stat.tile([128, 8], f32)
        for p_, (j0_, j1_) in enumerate(BATCHES):
            js = list(range(j0_, j1_))
            mxp = statl.tile([128, 3 * T], f32, tag="mxp")
            q2p = statl.tile([128, 3 * T], f32, tag="q2p")
            for ji, j in enumerate(js):
                for t in range(T):
                    pc = psum.tile([128, NCHUNK], f32, tag="mm")
                    for k in range(4):
                        nc.tensor.matmul(
                            pc[:, k * 512:(k + 1) * 512],
                            sb_w2b[:, j * 128:(j + 1) * 128],
                            h1[:, t * NCHUNK + k * 512: t * NCHUNK + (k + 1) * 512],
                        )
                    sq = scr.tile([128, NCHUNK], f32, tag="sq")
                    nc.scalar.activation(
                        sq[:], pc[:], AF.Square,
                        accum_out=q2p[:, ji * T + t:ji * T + t + 1],
                    )
                    nc.vector.reduce_max(
                        mxp[:, ji * T + t:ji * T + t + 1], pc[:], axis=AX.X)
            # batched stats: [sh1 (first batch only) | q2 sums for the batch]
            extra = 1 if p_ == 0 else 0
            nb = extra + len(js)
            stj = statl.tile([128, 4], f32, tag="stj")
            if extra:
                nc.vector.tensor_copy(stj[:, 0:1], sh1[:])
            for ji in range(len(js)):
                nc.vector.reduce_sum(
                    stj[:, extra + ji:extra + ji + 1],
                    q2p[:, ji * T:(ji + 1) * T], axis=AX.X)
            red = allreduce(stj[:, 0:nb], 128, nb, f"q2_{p_}")
            if extra:
                # mean2[:, j] = (W2 @ sum_h1)_chunk_j / NTOT
                sh1hat = stat.tile([128, 1], f32)
                nc.vector.tensor_copy(sh1hat[:], red[:, 0:1])
                for j in range(8):
                    pm = psum.tile([128, 1], f32, tag="mm")
                    nc.tensor.matmul(
                        pm[:], sb_w2t[:, j * 128:(j + 1) * 128], sh1hat[:])
                    nc.scalar.mul(m2[:, j:j + 1], pm[:], 1.0 / NTOT)
            # affine-transform the channel maxes + write 8 MB per chunk
            for ji, j in enumerate(js):
                mx = statl.tile([128, 1], f32, tag="mx")
                nc.vector.reduce_max(mx[:], mxp[:, ji * T:(ji + 1) * T], axis=AX.X)
                wv = statl.tile([128, 4], f32, tag="wv")
                # var = E[h2^2] - mean^2 ; rstd
                nc.scalar.mul(wv[:, 0:1], red[:, extra + ji:extra + ji + 1], 1.0 / NTOT)
                nc.vector.tensor_mul(wv[:, 1:2], m2[:, j:j + 1], m2[:, j:j + 1])
                nc.vector.tensor_sub(wv[:, 1:2], wv[:, 0:1], wv[:, 1:2])
                nc.scalar.activation(wv[:, 2:3], wv[:, 1:2], AF.Sqrt, bias=sb_eps[:, 0:1])
                nc.vector.reciprocal(wv[:, 3:4], wv[:, 2:3])
                # scale2 = g2*rstd ; shift2 = be2 - mean*scale2 ;
                # gfeat = scale2*max + shift2
                sc2 = statl.tile([128, 3], f32, tag="sc2")
                nc.vector.tensor_mul(sc2[:, 0:1], sb_g2[:, j:j + 1], wv[:, 3:4])
                nc.vector.tensor_mul(sc2[:, 1:2], m2[:, j:j + 1], sc2[:, 0:1])
                nc.vector.tensor_sub(sc2[:, 1:2], sb_be2[:, j:j + 1], sc2[:, 1:2])
                gf = statl.tile([128, 1], f32, tag="gf")
                nc.vector.tensor_scalar(
                    out=gf[:], in0=mx[:], scalar1=sc2[:, 0:1],
                    scalar2=sc2[:, 1:2], op0=OP.mult, op1=OP.add,
                )
                # broadcast gfeat along the free dim; the 8 MB write is split
                # column-wise over the three DMA queues (rotating per j)
                bc = bcp.tile([128, NCHUNK], f32, tag="bc")
                nc.vector.tensor_copy(bc[:], gf[:, 0:1].to_broadcast([128, NCHUNK]))
                engs = [nc.sync, nc.scalar, nc.gpsimd]
                splits = [(0, 3), (3, 6), (6, 8)]
                for qi, (a, b_) in enumerate(splits):
                    eng = engs[(qi + j) % 3]
                    srcq = bc[:].unsqueeze(1).broadcast_to([128, b_ - a, NCHUNK])
                    eng.dma_start(
                        out[j * 128:(j + 1) * 128, a * NCHUNK:b_ * NCHUNK], srcq)


@functools.lru_cache(maxsize=1)
def build_program():
    import concourse.bacc as bacc
    import concourse.tile as tile
    from concourse import mybir

    f32 = mybir.dt.float32
    nc = bacc.Bacc(
        "TRN2", target_bir_lowering=False, debug=False, num_devices=NCORES
    )
    io = {
        "xT": nc.dram_tensor("xT", [3, N], f32, kind="ExternalInput").ap(),
        "w0t": nc.dram_tensor("w0t", [3, 64], f32, kind="ExternalInput").ap(),
        "w1t": nc.dram_tensor("w1t", [64, 128], f32, kind="ExternalInput").ap(),
        "w2t": nc.dram_tensor("w2t", [128, 1024], f32, kind="ExternalInput").ap(),
        "g0be0": nc.dram_tensor("g0be0", [64, 2], f32, kind="ExternalInput").ap(),
        "g1be1": nc.dram_tensor("g1be1", [128, 2], f32, kind="ExternalInput").ap(),
        "g2p": nc.dram_tensor("g2p", [128, 8], f32, kind="ExternalInput").ap(),
        "be2p": nc.dram_tensor("be2p", [128, 8], f32, kind="ExternalInput").ap(),
        "out": nc.dram_tensor("out", [1088, N], f32, kind="ExternalOutput").ap(),
    }
    with tile.TileContext(nc) as tc:
        _body(nc, tc, io)
    nc.compile()
    return nc


def make_in_maps(x, W0, W1, W2, g0, be0, g1, be1, g2, be2):
    x = np.asarray(x, np.float32)
    shared = {
        "w0t": np.ascontiguousarray(np.asarray(W0, np.float32).T),
        "w1t": np.ascontiguousarray(np.asarray(W1, np.float32).T),
        "w2t": np.ascontiguousarray(np.asarray(W2, np.float32).T),
        "g0be0": np.ascontiguousarray(
            np.stack([np.asarray(g0, np.float32), np.asarray(be0, np.float32)], 1)),
        "g1be1": np.ascontiguousarray(
            np.stack([np.asarray(g1, np.float32), np.asarray(be1, np.float32)], 1)),
        "g2p": np.ascontiguousarray(np.asarray(g2, np.float32).reshape(8, 128).T),
        "be2p": np.ascontiguousarray(np.asarray(be2, np.float32).reshape(8, 128).T),
    }
    return [
        {"xT": np.ascontiguousarray(x[i].T), **shared} for i in range(NCORES)
    ]


def kernel(x, W0, b0, g0, be0, W1, b1, g1, be1, W2, b2, g2, be2):
    """Full inputs in, full output out.  b0/b1/b2 cancel inside BN."""
    from concourse.bass_utils import run_bass_kernel_spmd

    nc = build_program()
    in_maps = make_in_maps(x, W0, W1, W2, g0, be0, g1, be1, g2, be2)
    res = run_bass_kernel_spmd(nc, in_maps, core_ids=list(range(NCORES)))
    return np.stack([res.results[i]["out"] for i in range(NCORES)], axis=0)
